# revision 15
# baseline (speedup 1.0000x reference)
"""MoE top-2 routing kernel for Trainium2 (8 NeuronCores, expert-parallel).

Strategy
--------
Host (cheap, 16384x4-sized math): router logits, sinkhorn, top-2 indices and
sigmoid gates — computed with jax on CPU, replicating the reference bitwise.
Tokens are permuted by expert on the host while sharding: each of the 8 cores
owns half of one expert's (token, gate) list plus that expert's W1/W2 (bf16).

Device (the heavy ~17 GFLOP/core): dense FFN over the pre-gathered tokens in
feature-major layout, weight-stationary matmuls from SBUF:
    h1T = silu(W1_chunk.T @ xT)      [F-major]
    y   = gate * (h1T_chunk.T @ W2)  [token-major out]
Host scatter-adds the two expert contributions per token (no duplicates per
core, so fancy-index += is safe).
"""
import sys
import types

import numpy as np
import ml_dtypes

H = 512
F = 2048
E = 4
P = 128
PANEL = 512
NCORES = 8
T_TOTAL = 16384
DEFAULT_C = 4224  # rows (token,expert pairs) per core, multiple of 128

_BF16 = ml_dtypes.bfloat16


# ---------------------------------------------------------------------------
# compat shims (axon image): NTFF hook module + core_v3 drain-wait splitting
# ---------------------------------------------------------------------------
_COMPAT_DONE = False


def _install_compat():
    global _COMPAT_DONE
    if _COMPAT_DONE:
        return
    if "antenv.axon_hooks" not in sys.modules:
        mod = types.ModuleType("antenv.axon_hooks")
        try:
            from trn_agent_boot.trn_boot import _ntff_profile_via_ctypes
            _hook = _ntff_profile_via_ctypes("/opt/axon/libaxon_pjrt.so")
        except Exception:
            _hook = None
        mod.get_axon_ntff_profile_hook = lambda: _hook
        mod.set_axon_ntff_profile_hook = lambda h: None
        sys.modules["antenv.axon_hooks"] = mod

    import concourse.mybir as mybir
    import concourse.tile as tile
    from bass_rust import VectorClock, ScopedClock, N_PROCS

    if not getattr(tile.TileContext._add_instruction, "_split_waits_patch", False):
        # This walrus build accepts at most ONE sync wait per instruction
        # ("Too many sync wait commands"). Split extras onto single-wait
        # nops on the same engine, inserted immediately before. Stalling the
        # engine at the same program point is strictly stronger than the
        # per-instruction wait, and every waited-on producer is issued
        # earlier in program order, so this cannot deadlock.
        _orig_add = tile.TileContext._add_instruction

        def _add_instruction(self, inst):
            si = inst.sync_info
            if si is not None and si.on_wait and len(si.on_wait) > 1:
                waits = list(si.on_wait)
                for w in waits[:-1]:
                    nop = mybir.InstNoOp(
                        name=self.nc.get_next_instruction_name()
                    )
                    nop.engine = inst.engine
                    nop.sync_info = mybir.SyncInfo(on_wait=[w], on_update=[])
                    _orig_add(self, nop)
                inst.sync_info = mybir.SyncInfo(
                    on_wait=[waits[-1]], on_update=list(si.on_update or [])
                )
            _orig_add(self, inst)

        _add_instruction._split_waits_patch = True
        tile.TileContext._add_instruction = _add_instruction

    if not getattr(tile.TileContext._drain_and_barrier, "_split_waits_patch", False):
        # core_v3 walrus rejects a Drain carrying >1 sync wait ("Too many sync
        # wait commands"); put each wait on its own in-order SP nop instead.
        def _drain_and_barrier(self, tick_clock, wait_clock):
            nc = self.nc
            gc = tick_clock.global_clock
            for p in range(N_PROCS):
                t = gc[p]
                if t == 0:
                    continue
                vc = VectorClock([t if i == p else 0 for i in range(N_PROCS)])
                n = nc.sync.nop()
                wait_clock.add_sem_waits(n.ins, ScopedClock({None: vc}))
            nc.sync.drain()
            nc.all_engine_barrier()
            popped = nc._tile_sem_poison_stack.pop()
            assert popped is self._sem_poison
            nc.clear_and_free_semaphores(list(self.sems.allocated().values()))
            nc.all_engine_barrier()

        _drain_and_barrier._split_waits_patch = True
        tile.TileContext._drain_and_barrier = _drain_and_barrier

    from concourse import bass_utils
    bass_utils.upload_artifacts = lambda tmpdir: tmpdir
    _COMPAT_DONE = True


# ---------------------------------------------------------------------------
# host routing — bitwise replication of the reference (jax on CPU)
# ---------------------------------------------------------------------------
def _route(xt_f32, w_router):
    import jax
    import jax.numpy as jnp
    from jax import lax

    cpu = jax.devices("cpu")[0]

    def sinkhorn(cost, tol=1e-4):
        cost = jnp.exp(cost)
        T, E_ = cost.shape
        eps = 1e-8

        def cond(state):
            _, _, err = state
            return err > tol

        def body(state):
            d0, d1, _ = state
            d0n = (1.0 / T) / (cost @ d1 + eps)
            d1n = (1.0 / E_) / (d0n @ cost + eps)
            return (d0n, d1n, jnp.mean(jnp.abs(d1 - d1n)))

        init = (jnp.ones((T,), cost.dtype), jnp.ones((E_,), cost.dtype),
                jnp.asarray(1e9, cost.dtype))
        d0, d1, _ = lax.while_loop(cond, body, init)
        return d1 * cost * d0[:, None]

    with jax.default_device(cpu):
        xt_j = jnp.asarray(xt_f32)
        logits = xt_j @ jnp.asarray(w_router)
        norm = sinkhorn(logits.astype(jnp.float32))
        _, indices = lax.top_k(norm, 2)
        scores = jnp.take_along_axis(jax.nn.sigmoid(logits), indices, axis=1)
        return np.asarray(indices), np.asarray(scores)


# ---------------------------------------------------------------------------
# device kernel
# ---------------------------------------------------------------------------
_BUILD_CACHE = {}
LAST_EXEC_NS = None
LAST_RESULTS = None


def _build_nc(C):
    """Bass program for one core: dense FFN over C pre-gathered rows."""
    import concourse.bass as bass
    import concourse.mybir as mybir
    import concourse.tile as tile

    assert C % P == 0
    KC = H // P            # 4  k-chunks over hidden
    FC = F // P            # 16 f-chunks over ffn
    bf16 = mybir.dt.bfloat16
    f32 = mybir.dt.float32

    # token panels: full PANELs plus one remainder panel (multiple of 128)
    panels = []
    off = 0
    while off < C:
        w = min(PANEL, C - off)
        panels.append((off, w))
        off += w

    nc = bass.Bass()
    xt_d = nc.dram_tensor("xt", [P, KC, C], bf16, kind="ExternalInput")
    w1_d = nc.dram_tensor("w1", [P, KC, F], bf16, kind="ExternalInput")
    w2_d = nc.dram_tensor("w2", [P, FC, H], bf16, kind="ExternalInput")
    g_d = nc.dram_tensor("g", [P, C // P], f32, kind="ExternalInput")
    y_d = nc.dram_tensor("y", [C // P, P, H], f32, kind="ExternalOutput")

    with tile.TileContext(nc) as tc:
        with (
            tc.tile_pool(name="wpool", bufs=1) as wp,
            tc.tile_pool(name="xpool", bufs=4) as xp,
            tc.tile_pool(name="hpool", bufs=3) as hp,
            tc.tile_pool(name="opool", bufs=4) as op,
            tc.tile_pool(name="psum", bufs=4, space="PSUM") as pp,
        ):
            # HAM warm-up: ~16 dummy matmuls on zeroed SBUF while the input
            # DMAs are in flight. The PE clock gate needs ~3.4us of sustained
            # activity to open (1.2 -> 2.4 GHz); burn that during the startup
            # DMA window instead of during the first real matmuls.
            warm_sb = wp.tile([P, PANEL], bf16)
            nc.vector.memset(warm_sb, 0)
            warm_ps = pp.tile([P, PANEL], f32, tag="ps1")
            for _ in range(16):
                nc.tensor.matmul(
                    warm_ps, warm_sb[:, :P], warm_sb, start=True, stop=True
                )

            # first panel's tokens before the weights: PE needs x0 + w1 to
            # start; chunked DMAs land on parallel HW queues.
            p0_off, p0_w = panels[0]
            x0_sb = xp.tile([P, KC, PANEL], bf16, tag="x")
            x0_dma = None
            for kc in range(KC):
                eng = nc.sync if kc % 2 == 0 else nc.gpsimd
                x0_dma = eng.dma_start(
                    out=x0_sb[:, kc, :p0_w],
                    in_=xt_d[:, kc, p0_off:p0_off + p0_w],
                )

            w1_sb = wp.tile([P, KC, F], bf16)
            for kc in range(KC):
                for hf in range(2):
                    eng = nc.sync if hf == 0 else nc.gpsimd
                    eng.dma_start(
                        out=w1_sb[:, kc, hf * (F // 2):(hf + 1) * (F // 2)],
                        in_=w1_d[:, kc, hf * (F // 2):(hf + 1) * (F // 2)],
                    )
            # w2 isn't needed until the first phase-2 (~28us in); keep its
            # 2MB off the startup queues until the first x panel has landed.
            w2_sb = wp.tile([P, FC, H], bf16)
            for q in range(4):
                w2_dma = nc.sync.dma_start(
                    out=w2_sb[:, q * 4:(q + 1) * 4, :],
                    in_=w2_d[:, q * 4:(q + 1) * 4, :],
                )
                tile.add_dep_helper(
                    w2_dma.ins, x0_dma.ins, sync=True,
                    reason="delay w2 load past x0",
                )
            g_sb = wp.tile([P, C // P], f32)
            nc.sync.dma_start(out=g_sb, in_=g_d[:, :])

            for ip, (poff, pw) in enumerate(panels):
                tch_n = pw // P
                if ip == 0:
                    x_sb = x0_sb
                else:
                    x_sb = xp.tile([P, KC, PANEL], bf16, tag="x")
                    nc.sync.dma_start(
                        out=x_sb[:, :, :pw], in_=xt_d[:, :, poff:poff + pw]
                    )
                h1_sb = hp.tile([P, FC, PANEL], bf16, tag="h1")
                for fc in range(FC):
                    ps = pp.tile([P, PANEL], f32, tag="ps1")
                    for kc in range(KC):
                        nc.tensor.matmul(
                            ps[:, :pw],
                            w1_sb[:, kc, fc * P:(fc + 1) * P],
                            x_sb[:, kc, :pw],
                            start=(kc == 0),
                            stop=(kc == KC - 1),
                        )
                    nc.scalar.activation(
                        out=h1_sb[:, fc, :pw], in_=ps[:, :pw],
                        func=mybir.ActivationFunctionType.Silu,
                    )
                for tch in range(tch_n):
                    ps2 = pp.tile([P, H], f32, tag="ps2")
                    for fc in range(FC):
                        nc.tensor.matmul(
                            ps2,
                            h1_sb[:, fc, tch * P:(tch + 1) * P],
                            w2_sb[:, fc, :],
                            start=(fc == 0),
                            stop=(fc == FC - 1),
                        )
                    o_sb = op.tile([P, H], f32, tag="o")
                    j = poff // P + tch
                    nc.vector.tensor_scalar_mul(o_sb, ps2, g_sb[:, j:j + 1])
                    nc.sync.dma_start(out=y_d[j], in_=o_sb)
    return nc


def _pack_core(xt_f32, toks, gates, w1_e_bf, w2_e_bf, C):
    n = len(toks)
    xr = np.zeros((C, H), _BF16)
    xr[:n] = xt_f32[toks].astype(_BF16)
    # [C,H] -> [H,C] -> [KC,P,C] -> [P,KC,C]
    xt_pack = np.ascontiguousarray(
        xr.T.reshape(H // P, P, C).transpose(1, 0, 2)
    )
    g = np.zeros((C,), np.float32)
    g[:n] = gates
    g_pack = np.ascontiguousarray(g.reshape(C // P, P).T)
    return {"xt": xt_pack, "w1": w1_e_bf, "w2": w2_e_bf, "g": g_pack}


def kernel(input, w_router, w1, w2):
    global LAST_EXEC_NS, LAST_RESULTS
    import os

    _install_compat()
    from concourse.bass_utils import run_bass_kernel_spmd

    x = np.asarray(input, dtype=np.float32)
    w_router = np.asarray(w_router, dtype=np.float32)
    w1 = np.asarray(w1, dtype=np.float32)
    w2 = np.asarray(w2, dtype=np.float32)
    s, b, h = x.shape
    T = s * b
    xt = np.ascontiguousarray(x.reshape(T, h))

    indices, scores = _route(xt, w_router)

    # per-expert (token, gate) lists
    tok_lists = []
    gate_lists = []
    for e in range(E):
        toks = []
        gs = []
        for k in range(2):
            sel = np.nonzero(indices[:, k] == e)[0]
            toks.append(sel)
            gs.append(scores[sel, k])
        tok_lists.append(np.concatenate(toks))
        gate_lists.append(np.concatenate(gs).astype(np.float32))

    max_half = max((len(t) + 1) // 2 for t in tok_lists)
    C = max(DEFAULT_C, ((max_half + P - 1) // P) * P)

    if C not in _BUILD_CACHE:
        _BUILD_CACHE[C] = _build_nc(C)
    nc = _BUILD_CACHE[C]

    # weights per expert, packed [P, KC, F] / [P, FC, H] bf16
    w1_packs = [
        np.ascontiguousarray(
            w1[e].astype(_BF16).reshape(H // P, P, F).transpose(1, 0, 2)
        )
        for e in range(E)
    ]
    w2_packs = [
        np.ascontiguousarray(
            w2[e].astype(_BF16).reshape(F // P, P, H).transpose(1, 0, 2)
        )
        for e in range(E)
    ]

    in_maps = []
    core_toks = []
    for c in range(NCORES):
        e = c // 2
        toks_e = tok_lists[e]
        gates_e = gate_lists[e]
        half = (len(toks_e) + 1) // 2
        if c % 2 == 0:
            toks, gs = toks_e[:half], gates_e[:half]
        else:
            toks, gs = toks_e[half:], gates_e[half:]
        assert len(toks) <= C
        core_toks.append(toks)
        in_maps.append(_pack_core(xt, toks, gs, w1_packs[e], w2_packs[e], C))

    trace = bool(int(os.environ.get("BASS_MOE_TRACE", "0")))
    # The axon/NRT path can throw a transient NRT_EXEC_UNIT_UNRECOVERABLE;
    # the dispatch is a pure function of in_maps, so retrying is safe.
    last_err = None
    for attempt in range(3):
        try:
            res = run_bass_kernel_spmd(
                nc, in_maps, list(range(NCORES)), trace=trace
            )
            break
        except Exception as e:
            last_err = e
            print(f"kernel: device run attempt {attempt} failed: {e}",
                  file=sys.stderr)
            import time as _time
            _time.sleep(2.0)
    else:
        raise last_err
    LAST_EXEC_NS = res.exec_time_ns
    LAST_RESULTS = res

    out = np.zeros((T, H), np.float32)
    for c in range(NCORES):
        y = res.results[c]["y"].reshape(-1, H)
        toks = core_toks[c]
        out[toks] += y[: len(toks)]
    return out.reshape(s, b, h)


# revision 16
# speedup vs baseline: 1.0200x; 1.0200x over previous
"""MoE top-2 routing kernel for Trainium2 (8 NeuronCores, expert-parallel).

Strategy
--------
Host (cheap, 16384x4-sized math): router logits, sinkhorn, top-2 indices and
sigmoid gates — computed with jax on CPU, replicating the reference bitwise.
Tokens are permuted by expert on the host while sharding: each of the 8 cores
owns half of one expert's (token, gate) list plus that expert's W1/W2 (bf16).

Device (the heavy ~17 GFLOP/core): dense FFN over the pre-gathered tokens in
feature-major layout, weight-stationary matmuls from SBUF:
    h1T = silu(W1_chunk.T @ xT)      [F-major]
    y   = gate * (h1T_chunk.T @ W2)  [token-major out]
Host scatter-adds the two expert contributions per token (no duplicates per
core, so fancy-index += is safe).
"""
import sys
import types

import numpy as np
import ml_dtypes

H = 512
F = 2048
E = 4
P = 128
PANEL = 512
NCORES = 8
T_TOTAL = 16384
DEFAULT_C = 4224  # rows (token,expert pairs) per core, multiple of 128

_BF16 = ml_dtypes.bfloat16


# ---------------------------------------------------------------------------
# compat shims (axon image): NTFF hook module + core_v3 drain-wait splitting
# ---------------------------------------------------------------------------
_COMPAT_DONE = False


def _install_compat():
    global _COMPAT_DONE
    if _COMPAT_DONE:
        return
    if "antenv.axon_hooks" not in sys.modules:
        mod = types.ModuleType("antenv.axon_hooks")
        try:
            from trn_agent_boot.trn_boot import _ntff_profile_via_ctypes
            _hook = _ntff_profile_via_ctypes("/opt/axon/libaxon_pjrt.so")
        except Exception:
            _hook = None
        mod.get_axon_ntff_profile_hook = lambda: _hook
        mod.set_axon_ntff_profile_hook = lambda h: None
        sys.modules["antenv.axon_hooks"] = mod

    import concourse.mybir as mybir
    import concourse.tile as tile
    from bass_rust import VectorClock, ScopedClock, N_PROCS

    if not getattr(tile.TileContext._add_instruction, "_split_waits_patch", False):
        # This walrus build accepts at most ONE sync wait per instruction
        # ("Too many sync wait commands"). Split extras onto single-wait
        # nops on the same engine, inserted immediately before. Stalling the
        # engine at the same program point is strictly stronger than the
        # per-instruction wait, and every waited-on producer is issued
        # earlier in program order, so this cannot deadlock.
        _orig_add = tile.TileContext._add_instruction

        def _add_instruction(self, inst):
            si = inst.sync_info
            if si is not None and si.on_wait and len(si.on_wait) > 1:
                waits = list(si.on_wait)
                for w in waits[:-1]:
                    nop = mybir.InstNoOp(
                        name=self.nc.get_next_instruction_name()
                    )
                    nop.engine = inst.engine
                    nop.sync_info = mybir.SyncInfo(on_wait=[w], on_update=[])
                    _orig_add(self, nop)
                inst.sync_info = mybir.SyncInfo(
                    on_wait=[waits[-1]], on_update=list(si.on_update or [])
                )
            _orig_add(self, inst)

        _add_instruction._split_waits_patch = True
        tile.TileContext._add_instruction = _add_instruction

    if not getattr(tile.TileContext._drain_and_barrier, "_split_waits_patch", False):
        # core_v3 walrus rejects a Drain carrying >1 sync wait ("Too many sync
        # wait commands"); put each wait on its own in-order SP nop instead.
        def _drain_and_barrier(self, tick_clock, wait_clock):
            nc = self.nc
            gc = tick_clock.global_clock
            for p in range(N_PROCS):
                t = gc[p]
                if t == 0:
                    continue
                vc = VectorClock([t if i == p else 0 for i in range(N_PROCS)])
                n = nc.sync.nop()
                wait_clock.add_sem_waits(n.ins, ScopedClock({None: vc}))
            nc.sync.drain()
            nc.all_engine_barrier()
            popped = nc._tile_sem_poison_stack.pop()
            assert popped is self._sem_poison
            nc.clear_and_free_semaphores(list(self.sems.allocated().values()))
            nc.all_engine_barrier()

        _drain_and_barrier._split_waits_patch = True
        tile.TileContext._drain_and_barrier = _drain_and_barrier

    from concourse import bass_utils
    bass_utils.upload_artifacts = lambda tmpdir: tmpdir
    _COMPAT_DONE = True


# ---------------------------------------------------------------------------
# host routing — bitwise replication of the reference (jax on CPU)
# ---------------------------------------------------------------------------
def _route(xt_f32, w_router):
    import jax
    import jax.numpy as jnp
    from jax import lax

    cpu = jax.devices("cpu")[0]

    def sinkhorn(cost, tol=1e-4):
        cost = jnp.exp(cost)
        T, E_ = cost.shape
        eps = 1e-8

        def cond(state):
            _, _, err = state
            return err > tol

        def body(state):
            d0, d1, _ = state
            d0n = (1.0 / T) / (cost @ d1 + eps)
            d1n = (1.0 / E_) / (d0n @ cost + eps)
            return (d0n, d1n, jnp.mean(jnp.abs(d1 - d1n)))

        init = (jnp.ones((T,), cost.dtype), jnp.ones((E_,), cost.dtype),
                jnp.asarray(1e9, cost.dtype))
        d0, d1, _ = lax.while_loop(cond, body, init)
        return d1 * cost * d0[:, None]

    with jax.default_device(cpu):
        xt_j = jnp.asarray(xt_f32)
        logits = xt_j @ jnp.asarray(w_router)
        norm = sinkhorn(logits.astype(jnp.float32))
        _, indices = lax.top_k(norm, 2)
        scores = jnp.take_along_axis(jax.nn.sigmoid(logits), indices, axis=1)
        return np.asarray(indices), np.asarray(scores)


# ---------------------------------------------------------------------------
# device kernel
# ---------------------------------------------------------------------------
_BUILD_CACHE = {}
LAST_EXEC_NS = None
LAST_RESULTS = None


def _build_nc(C):
    """Bass program for one core: dense FFN over C pre-gathered rows."""
    import concourse.bass as bass
    import concourse.mybir as mybir
    import concourse.tile as tile

    assert C % P == 0
    KC = H // P            # 4  k-chunks over hidden
    FC = F // P            # 16 f-chunks over ffn
    bf16 = mybir.dt.bfloat16
    f32 = mybir.dt.float32

    # token panels: full PANELs plus one remainder panel (multiple of 128)
    panels = []
    off = 0
    while off < C:
        w = min(PANEL, C - off)
        panels.append((off, w))
        off += w

    nc = bass.Bass()
    xt_d = nc.dram_tensor("xt", [P, KC, C], bf16, kind="ExternalInput")
    w1_d = nc.dram_tensor("w1", [P, KC, F], bf16, kind="ExternalInput")
    w2_d = nc.dram_tensor("w2", [P, FC, H], bf16, kind="ExternalInput")
    g_d = nc.dram_tensor("g", [P, C // P], f32, kind="ExternalInput")
    y_d = nc.dram_tensor("y", [C // P, P, H], f32, kind="ExternalOutput")

    with tile.TileContext(nc) as tc:
        with (
            tc.tile_pool(name="wpool", bufs=1) as wp,
            tc.tile_pool(name="xpool", bufs=4) as xp,
            tc.tile_pool(name="hpool", bufs=3) as hp,
            tc.tile_pool(name="opool", bufs=4) as op,
            tc.tile_pool(name="psum", bufs=4, space="PSUM") as pp,
        ):
            # HAM warm-up: ~16 dummy matmuls on zeroed SBUF while the input
            # DMAs are in flight. The PE clock gate needs ~3.4us of sustained
            # activity to open (1.2 -> 2.4 GHz); burn that during the startup
            # DMA window instead of during the first real matmuls.
            warm_sb = wp.tile([P, PANEL], bf16)
            nc.vector.memset(warm_sb, 0)
            warm_ps = pp.tile([P, PANEL], f32, tag="ps1")
            for _ in range(16):
                nc.tensor.matmul(
                    warm_ps, warm_sb[:, :P], warm_sb, start=True, stop=True
                )

            # first panel's tokens before the weights: PE needs x0 + w1 to
            # start; chunked DMAs land on parallel HW queues.
            p0_off, p0_w = panels[0]
            x0_sb = xp.tile([P, KC, PANEL], bf16, tag="x")
            x0_dma = None
            for kc in range(KC):
                x0_dma = nc.sync.dma_start(
                    out=x0_sb[:, kc, :p0_w],
                    in_=xt_d[:, kc, p0_off:p0_off + p0_w],
                )

            w1_sb = wp.tile([P, KC, F], bf16)
            for kc in range(KC):
                for hf in range(2):
                    nc.sync.dma_start(
                        out=w1_sb[:, kc, hf * (F // 2):(hf + 1) * (F // 2)],
                        in_=w1_d[:, kc, hf * (F // 2):(hf + 1) * (F // 2)],
                    )
            # w2 isn't needed until the first phase-2 (~28us in); keep its
            # 2MB off the startup queues until the first x panel has landed.
            w2_sb = wp.tile([P, FC, H], bf16)
            for q in range(4):
                w2_dma = nc.sync.dma_start(
                    out=w2_sb[:, q * 4:(q + 1) * 4, :],
                    in_=w2_d[:, q * 4:(q + 1) * 4, :],
                )
                tile.add_dep_helper(
                    w2_dma.ins, x0_dma.ins, sync=True,
                    reason="delay w2 load past x0",
                )
            g_sb = wp.tile([P, C // P], f32)
            nc.sync.dma_start(out=g_sb, in_=g_d[:, :])

            for ip, (poff, pw) in enumerate(panels):
                tch_n = pw // P
                if ip == 0:
                    x_sb = x0_sb
                else:
                    x_sb = xp.tile([P, KC, PANEL], bf16, tag="x")
                    nc.sync.dma_start(
                        out=x_sb[:, :, :pw], in_=xt_d[:, :, poff:poff + pw]
                    )
                h1_sb = hp.tile([P, FC, PANEL], bf16, tag="h1")
                for fc in range(FC):
                    ps = pp.tile([P, PANEL], f32, tag="ps1")
                    for kc in range(KC):
                        nc.tensor.matmul(
                            ps[:, :pw],
                            w1_sb[:, kc, fc * P:(fc + 1) * P],
                            x_sb[:, kc, :pw],
                            start=(kc == 0),
                            stop=(kc == KC - 1),
                        )
                    nc.scalar.activation(
                        out=h1_sb[:, fc, :pw], in_=ps[:, :pw],
                        func=mybir.ActivationFunctionType.Silu,
                    )
                for tch in range(tch_n):
                    ps2 = pp.tile([P, H], f32, tag="ps2")
                    for fc in range(FC):
                        nc.tensor.matmul(
                            ps2,
                            h1_sb[:, fc, tch * P:(tch + 1) * P],
                            w2_sb[:, fc, :],
                            start=(fc == 0),
                            stop=(fc == FC - 1),
                        )
                    o_sb = op.tile([P, H], f32, tag="o")
                    j = poff // P + tch
                    nc.vector.tensor_scalar_mul(o_sb, ps2, g_sb[:, j:j + 1])
                    nc.sync.dma_start(out=y_d[j], in_=o_sb)
    return nc


def _pack_core(xt_f32, toks, gates, w1_e_bf, w2_e_bf, C):
    n = len(toks)
    xr = np.zeros((C, H), _BF16)
    xr[:n] = xt_f32[toks].astype(_BF16)
    # [C,H] -> [H,C] -> [KC,P,C] -> [P,KC,C]
    xt_pack = np.ascontiguousarray(
        xr.T.reshape(H // P, P, C).transpose(1, 0, 2)
    )
    g = np.zeros((C,), np.float32)
    g[:n] = gates
    g_pack = np.ascontiguousarray(g.reshape(C // P, P).T)
    return {"xt": xt_pack, "w1": w1_e_bf, "w2": w2_e_bf, "g": g_pack}


def kernel(input, w_router, w1, w2):
    global LAST_EXEC_NS, LAST_RESULTS
    import os

    _install_compat()
    from concourse.bass_utils import run_bass_kernel_spmd

    x = np.asarray(input, dtype=np.float32)
    w_router = np.asarray(w_router, dtype=np.float32)
    w1 = np.asarray(w1, dtype=np.float32)
    w2 = np.asarray(w2, dtype=np.float32)
    s, b, h = x.shape
    T = s * b
    xt = np.ascontiguousarray(x.reshape(T, h))

    indices, scores = _route(xt, w_router)

    # per-expert (token, gate) lists
    tok_lists = []
    gate_lists = []
    for e in range(E):
        toks = []
        gs = []
        for k in range(2):
            sel = np.nonzero(indices[:, k] == e)[0]
            toks.append(sel)
            gs.append(scores[sel, k])
        tok_lists.append(np.concatenate(toks))
        gate_lists.append(np.concatenate(gs).astype(np.float32))

    max_half = max((len(t) + 1) // 2 for t in tok_lists)
    C = max(DEFAULT_C, ((max_half + P - 1) // P) * P)

    if C not in _BUILD_CACHE:
        _BUILD_CACHE[C] = _build_nc(C)
    nc = _BUILD_CACHE[C]

    # weights per expert, packed [P, KC, F] / [P, FC, H] bf16
    w1_packs = [
        np.ascontiguousarray(
            w1[e].astype(_BF16).reshape(H // P, P, F).transpose(1, 0, 2)
        )
        for e in range(E)
    ]
    w2_packs = [
        np.ascontiguousarray(
            w2[e].astype(_BF16).reshape(F // P, P, H).transpose(1, 0, 2)
        )
        for e in range(E)
    ]

    in_maps = []
    core_toks = []
    for c in range(NCORES):
        e = c // 2
        toks_e = tok_lists[e]
        gates_e = gate_lists[e]
        half = (len(toks_e) + 1) // 2
        if c % 2 == 0:
            toks, gs = toks_e[:half], gates_e[:half]
        else:
            toks, gs = toks_e[half:], gates_e[half:]
        assert len(toks) <= C
        core_toks.append(toks)
        in_maps.append(_pack_core(xt, toks, gs, w1_packs[e], w2_packs[e], C))

    trace = bool(int(os.environ.get("BASS_MOE_TRACE", "0")))
    # The axon/NRT path can throw a transient NRT_EXEC_UNIT_UNRECOVERABLE;
    # the dispatch is a pure function of in_maps, so retrying is safe.
    last_err = None
    for attempt in range(3):
        try:
            res = run_bass_kernel_spmd(
                nc, in_maps, list(range(NCORES)), trace=trace
            )
            break
        except Exception as e:
            last_err = e
            print(f"kernel: device run attempt {attempt} failed: {e}",
                  file=sys.stderr)
            import time as _time
            _time.sleep(2.0)
    else:
        raise last_err
    LAST_EXEC_NS = res.exec_time_ns
    LAST_RESULTS = res

    out = np.zeros((T, H), np.float32)
    for c in range(NCORES):
        y = res.results[c]["y"].reshape(-1, H)
        toks = core_toks[c]
        out[toks] += y[: len(toks)]
    return out.reshape(s, b, h)


# revision 18
# speedup vs baseline: 1.0231x; 1.0030x over previous
"""MoE top-2 routing kernel for Trainium2 (8 NeuronCores, expert-parallel).

Strategy
--------
Host (cheap, 16384x4-sized math): router logits, sinkhorn, top-2 indices and
sigmoid gates — computed with jax on CPU, replicating the reference bitwise.
Tokens are permuted by expert on the host while sharding: each of the 8 cores
owns half of one expert's (token, gate) list plus that expert's W1/W2 (bf16).

Device (the heavy ~17 GFLOP/core): dense FFN over the pre-gathered tokens in
feature-major layout, weight-stationary matmuls from SBUF:
    h1T = silu(W1_chunk.T @ xT)      [F-major]
    y   = gate * (h1T_chunk.T @ W2)  [token-major out]
Host scatter-adds the two expert contributions per token (no duplicates per
core, so fancy-index += is safe).
"""
import sys
import types

import numpy as np
import ml_dtypes

H = 512
F = 2048
E = 4
P = 128
PANEL = 512
NCORES = 8
T_TOTAL = 16384
DEFAULT_C = 4224  # rows (token,expert pairs) per core, multiple of 128

_BF16 = ml_dtypes.bfloat16


# ---------------------------------------------------------------------------
# compat shims (axon image): NTFF hook module + core_v3 drain-wait splitting
# ---------------------------------------------------------------------------
_COMPAT_DONE = False


def _install_compat():
    global _COMPAT_DONE
    if _COMPAT_DONE:
        return
    if "antenv.axon_hooks" not in sys.modules:
        mod = types.ModuleType("antenv.axon_hooks")
        try:
            from trn_agent_boot.trn_boot import _ntff_profile_via_ctypes
            _hook = _ntff_profile_via_ctypes("/opt/axon/libaxon_pjrt.so")
        except Exception:
            _hook = None
        mod.get_axon_ntff_profile_hook = lambda: _hook
        mod.set_axon_ntff_profile_hook = lambda h: None
        sys.modules["antenv.axon_hooks"] = mod

    import concourse.mybir as mybir
    import concourse.tile as tile
    from bass_rust import VectorClock, ScopedClock, N_PROCS

    if not getattr(tile.TileContext._add_instruction, "_split_waits_patch", False):
        # This walrus build accepts at most ONE sync wait per instruction
        # ("Too many sync wait commands"). Split extras onto single-wait
        # nops on the same engine, inserted immediately before. Stalling the
        # engine at the same program point is strictly stronger than the
        # per-instruction wait, and every waited-on producer is issued
        # earlier in program order, so this cannot deadlock.
        _orig_add = tile.TileContext._add_instruction

        def _add_instruction(self, inst):
            si = inst.sync_info
            if si is not None and si.on_wait and len(si.on_wait) > 1:
                waits = list(si.on_wait)
                for w in waits[:-1]:
                    nop = mybir.InstNoOp(
                        name=self.nc.get_next_instruction_name()
                    )
                    nop.engine = inst.engine
                    nop.sync_info = mybir.SyncInfo(on_wait=[w], on_update=[])
                    _orig_add(self, nop)
                inst.sync_info = mybir.SyncInfo(
                    on_wait=[waits[-1]], on_update=list(si.on_update or [])
                )
            _orig_add(self, inst)

        _add_instruction._split_waits_patch = True
        tile.TileContext._add_instruction = _add_instruction

    if not getattr(tile.TileContext._drain_and_barrier, "_split_waits_patch", False):
        # core_v3 walrus rejects a Drain carrying >1 sync wait ("Too many sync
        # wait commands"); put each wait on its own in-order SP nop instead.
        def _drain_and_barrier(self, tick_clock, wait_clock):
            nc = self.nc
            gc = tick_clock.global_clock
            for p in range(N_PROCS):
                t = gc[p]
                if t == 0:
                    continue
                vc = VectorClock([t if i == p else 0 for i in range(N_PROCS)])
                n = nc.sync.nop()
                wait_clock.add_sem_waits(n.ins, ScopedClock({None: vc}))
            nc.sync.drain()
            nc.all_engine_barrier()
            popped = nc._tile_sem_poison_stack.pop()
            assert popped is self._sem_poison
            nc.clear_and_free_semaphores(list(self.sems.allocated().values()))
            nc.all_engine_barrier()

        _drain_and_barrier._split_waits_patch = True
        tile.TileContext._drain_and_barrier = _drain_and_barrier

    from concourse import bass_utils
    bass_utils.upload_artifacts = lambda tmpdir: tmpdir
    _COMPAT_DONE = True


# ---------------------------------------------------------------------------
# host routing — bitwise replication of the reference (jax on CPU)
# ---------------------------------------------------------------------------
def _route(xt_f32, w_router):
    import jax
    import jax.numpy as jnp
    from jax import lax

    cpu = jax.devices("cpu")[0]

    def sinkhorn(cost, tol=1e-4):
        cost = jnp.exp(cost)
        T, E_ = cost.shape
        eps = 1e-8

        def cond(state):
            _, _, err = state
            return err > tol

        def body(state):
            d0, d1, _ = state
            d0n = (1.0 / T) / (cost @ d1 + eps)
            d1n = (1.0 / E_) / (d0n @ cost + eps)
            return (d0n, d1n, jnp.mean(jnp.abs(d1 - d1n)))

        init = (jnp.ones((T,), cost.dtype), jnp.ones((E_,), cost.dtype),
                jnp.asarray(1e9, cost.dtype))
        d0, d1, _ = lax.while_loop(cond, body, init)
        return d1 * cost * d0[:, None]

    with jax.default_device(cpu):
        xt_j = jnp.asarray(xt_f32)
        logits = xt_j @ jnp.asarray(w_router)
        norm = sinkhorn(logits.astype(jnp.float32))
        _, indices = lax.top_k(norm, 2)
        scores = jnp.take_along_axis(jax.nn.sigmoid(logits), indices, axis=1)
        return np.asarray(indices), np.asarray(scores)


# ---------------------------------------------------------------------------
# device kernel
# ---------------------------------------------------------------------------
_BUILD_CACHE = {}
LAST_EXEC_NS = None
LAST_RESULTS = None


def _build_nc(C):
    """Bass program for one core: dense FFN over C pre-gathered rows."""
    import concourse.bass as bass
    import concourse.mybir as mybir
    import concourse.tile as tile

    assert C % P == 0
    KC = H // P            # 4  k-chunks over hidden
    FC = F // P            # 16 f-chunks over ffn
    bf16 = mybir.dt.bfloat16
    f32 = mybir.dt.float32

    # token panels: full PANELs plus one remainder panel (multiple of 128)
    panels = []
    off = 0
    while off < C:
        w = min(PANEL, C - off)
        panels.append((off, w))
        off += w

    nc = bass.Bass()
    xt_d = nc.dram_tensor("xt", [P, KC, C], bf16, kind="ExternalInput")
    w1_d = nc.dram_tensor("w1", [P, KC, F], bf16, kind="ExternalInput")
    w2_d = nc.dram_tensor("w2", [P, FC, H], bf16, kind="ExternalInput")
    g_d = nc.dram_tensor("g", [P, C // P], f32, kind="ExternalInput")
    y_d = nc.dram_tensor("y", [C // P, P, H], f32, kind="ExternalOutput")

    with tile.TileContext(nc) as tc:
        with (
            tc.tile_pool(name="wpool", bufs=1) as wp,
            tc.tile_pool(name="xpool", bufs=4) as xp,
            tc.tile_pool(name="hpool", bufs=3) as hp,
            tc.tile_pool(name="opool", bufs=4) as op,
            tc.tile_pool(name="psum", bufs=4, space="PSUM") as pp,
        ):
            # HAM warm-up: ~16 dummy matmuls on zeroed SBUF while the input
            # DMAs are in flight. The PE clock gate needs ~3.4us of sustained
            # activity to open (1.2 -> 2.4 GHz); burn that during the startup
            # DMA window instead of during the first real matmuls.
            warm_sb = wp.tile([P, PANEL], bf16)
            nc.vector.memset(warm_sb, 0)
            warm_ps = pp.tile([P, PANEL], f32, tag="ps1")
            for _ in range(20):
                nc.tensor.matmul(
                    warm_ps, warm_sb[:, :P], warm_sb, start=True, stop=True
                )

            # first panel's tokens before the weights: PE needs x0 + w1 to
            # start; chunked DMAs land on parallel HW queues.
            p0_off, p0_w = panels[0]
            x0_sb = xp.tile([P, KC, PANEL], bf16, tag="x")
            x0_dma = None
            for kc in range(KC):
                x0_dma = nc.sync.dma_start(
                    out=x0_sb[:, kc, :p0_w],
                    in_=xt_d[:, kc, p0_off:p0_off + p0_w],
                )

            w1_sb = wp.tile([P, KC, F], bf16)
            for kc in range(KC):
                for hf in range(3):
                    lo = hf * 688
                    hi = min(F, lo + 688)
                    nc.sync.dma_start(
                        out=w1_sb[:, kc, lo:hi],
                        in_=w1_d[:, kc, lo:hi],
                    )
            # w2 isn't needed until the first phase-2 (~28us in); keep its
            # 2MB off the startup queues until the first x panel has landed.
            w2_sb = wp.tile([P, FC, H], bf16)
            for q in range(4):
                w2_dma = nc.sync.dma_start(
                    out=w2_sb[:, q * 4:(q + 1) * 4, :],
                    in_=w2_d[:, q * 4:(q + 1) * 4, :],
                )
                tile.add_dep_helper(
                    w2_dma.ins, x0_dma.ins, sync=True,
                    reason="delay w2 load past x0",
                )
            g_sb = wp.tile([P, C // P], f32)
            nc.sync.dma_start(out=g_sb, in_=g_d[:, :])

            for ip, (poff, pw) in enumerate(panels):
                tch_n = pw // P
                if ip == 0:
                    x_sb = x0_sb
                else:
                    x_sb = xp.tile([P, KC, PANEL], bf16, tag="x")
                    nc.sync.dma_start(
                        out=x_sb[:, :, :pw], in_=xt_d[:, :, poff:poff + pw]
                    )
                h1_sb = hp.tile([P, FC, PANEL], bf16, tag="h1")
                for fc in range(FC):
                    ps = pp.tile([P, PANEL], f32, tag="ps1")
                    for kc in range(KC):
                        nc.tensor.matmul(
                            ps[:, :pw],
                            w1_sb[:, kc, fc * P:(fc + 1) * P],
                            x_sb[:, kc, :pw],
                            start=(kc == 0),
                            stop=(kc == KC - 1),
                        )
                    nc.scalar.activation(
                        out=h1_sb[:, fc, :pw], in_=ps[:, :pw],
                        func=mybir.ActivationFunctionType.Silu,
                    )
                for tch in range(tch_n):
                    ps2 = pp.tile([P, H], f32, tag="ps2")
                    for fc in range(FC):
                        nc.tensor.matmul(
                            ps2,
                            h1_sb[:, fc, tch * P:(tch + 1) * P],
                            w2_sb[:, fc, :],
                            start=(fc == 0),
                            stop=(fc == FC - 1),
                        )
                    o_sb = op.tile([P, H], f32, tag="o")
                    j = poff // P + tch
                    nc.vector.tensor_scalar_mul(o_sb, ps2, g_sb[:, j:j + 1])
                    nc.sync.dma_start(out=y_d[j], in_=o_sb)
    return nc


def _pack_core(xt_f32, toks, gates, w1_e_bf, w2_e_bf, C):
    n = len(toks)
    xr = np.zeros((C, H), _BF16)
    xr[:n] = xt_f32[toks].astype(_BF16)
    # [C,H] -> [H,C] -> [KC,P,C] -> [P,KC,C]
    xt_pack = np.ascontiguousarray(
        xr.T.reshape(H // P, P, C).transpose(1, 0, 2)
    )
    g = np.zeros((C,), np.float32)
    g[:n] = gates
    g_pack = np.ascontiguousarray(g.reshape(C // P, P).T)
    return {"xt": xt_pack, "w1": w1_e_bf, "w2": w2_e_bf, "g": g_pack}


def kernel(input, w_router, w1, w2):
    global LAST_EXEC_NS, LAST_RESULTS
    import os

    _install_compat()
    from concourse.bass_utils import run_bass_kernel_spmd

    x = np.asarray(input, dtype=np.float32)
    w_router = np.asarray(w_router, dtype=np.float32)
    w1 = np.asarray(w1, dtype=np.float32)
    w2 = np.asarray(w2, dtype=np.float32)
    s, b, h = x.shape
    T = s * b
    xt = np.ascontiguousarray(x.reshape(T, h))

    indices, scores = _route(xt, w_router)

    # per-expert (token, gate) lists
    tok_lists = []
    gate_lists = []
    for e in range(E):
        toks = []
        gs = []
        for k in range(2):
            sel = np.nonzero(indices[:, k] == e)[0]
            toks.append(sel)
            gs.append(scores[sel, k])
        tok_lists.append(np.concatenate(toks))
        gate_lists.append(np.concatenate(gs).astype(np.float32))

    max_half = max((len(t) + 1) // 2 for t in tok_lists)
    C = max(DEFAULT_C, ((max_half + P - 1) // P) * P)

    if C not in _BUILD_CACHE:
        _BUILD_CACHE[C] = _build_nc(C)
    nc = _BUILD_CACHE[C]

    # weights per expert, packed [P, KC, F] / [P, FC, H] bf16
    w1_packs = [
        np.ascontiguousarray(
            w1[e].astype(_BF16).reshape(H // P, P, F).transpose(1, 0, 2)
        )
        for e in range(E)
    ]
    w2_packs = [
        np.ascontiguousarray(
            w2[e].astype(_BF16).reshape(F // P, P, H).transpose(1, 0, 2)
        )
        for e in range(E)
    ]

    in_maps = []
    core_toks = []
    for c in range(NCORES):
        e = c // 2
        toks_e = tok_lists[e]
        gates_e = gate_lists[e]
        half = (len(toks_e) + 1) // 2
        if c % 2 == 0:
            toks, gs = toks_e[:half], gates_e[:half]
        else:
            toks, gs = toks_e[half:], gates_e[half:]
        assert len(toks) <= C
        core_toks.append(toks)
        in_maps.append(_pack_core(xt, toks, gs, w1_packs[e], w2_packs[e], C))

    trace = bool(int(os.environ.get("BASS_MOE_TRACE", "0")))
    # The axon/NRT path can throw a transient NRT_EXEC_UNIT_UNRECOVERABLE;
    # the dispatch is a pure function of in_maps, so retrying is safe.
    last_err = None
    for attempt in range(3):
        try:
            res = run_bass_kernel_spmd(
                nc, in_maps, list(range(NCORES)), trace=trace
            )
            break
        except Exception as e:
            last_err = e
            print(f"kernel: device run attempt {attempt} failed: {e}",
                  file=sys.stderr)
            import time as _time
            _time.sleep(2.0)
    else:
        raise last_err
    LAST_EXEC_NS = res.exec_time_ns
    LAST_RESULTS = res

    out = np.zeros((T, H), np.float32)
    for c in range(NCORES):
        y = res.results[c]["y"].reshape(-1, H)
        toks = core_toks[c]
        out[toks] += y[: len(toks)]
    return out.reshape(s, b, h)


# revision 19
# speedup vs baseline: 1.0280x; 1.0048x over previous
"""MoE top-2 routing kernel for Trainium2 (8 NeuronCores, expert-parallel).

Strategy
--------
Host (cheap, 16384x4-sized math): router logits, sinkhorn, top-2 indices and
sigmoid gates — computed with jax on CPU, replicating the reference bitwise.
Tokens are permuted by expert on the host while sharding: each of the 8 cores
owns half of one expert's (token, gate) list plus that expert's W1/W2 (bf16).

Device (the heavy ~17 GFLOP/core): dense FFN over the pre-gathered tokens in
feature-major layout, weight-stationary matmuls from SBUF:
    h1T = silu(W1_chunk.T @ xT)      [F-major]
    y   = gate * (h1T_chunk.T @ W2)  [token-major out]
Host scatter-adds the two expert contributions per token (no duplicates per
core, so fancy-index += is safe).
"""
import sys
import types

import numpy as np
import ml_dtypes

H = 512
F = 2048
E = 4
P = 128
PANEL = 512
NCORES = 8
T_TOTAL = 16384
DEFAULT_C = 4224  # rows (token,expert pairs) per core, multiple of 128

_BF16 = ml_dtypes.bfloat16


# ---------------------------------------------------------------------------
# compat shims (axon image): NTFF hook module + core_v3 drain-wait splitting
# ---------------------------------------------------------------------------
_COMPAT_DONE = False


def _install_compat():
    global _COMPAT_DONE
    if _COMPAT_DONE:
        return
    if "antenv.axon_hooks" not in sys.modules:
        mod = types.ModuleType("antenv.axon_hooks")
        try:
            from trn_agent_boot.trn_boot import _ntff_profile_via_ctypes
            _hook = _ntff_profile_via_ctypes("/opt/axon/libaxon_pjrt.so")
        except Exception:
            _hook = None
        mod.get_axon_ntff_profile_hook = lambda: _hook
        mod.set_axon_ntff_profile_hook = lambda h: None
        sys.modules["antenv.axon_hooks"] = mod

    import concourse.mybir as mybir
    import concourse.tile as tile
    from bass_rust import VectorClock, ScopedClock, N_PROCS

    if not getattr(tile.TileContext._add_instruction, "_split_waits_patch", False):
        # This walrus build accepts at most ONE sync wait per instruction
        # ("Too many sync wait commands"). Split extras onto single-wait
        # nops on the same engine, inserted immediately before. Stalling the
        # engine at the same program point is strictly stronger than the
        # per-instruction wait, and every waited-on producer is issued
        # earlier in program order, so this cannot deadlock.
        _orig_add = tile.TileContext._add_instruction

        def _add_instruction(self, inst):
            si = inst.sync_info
            if si is not None and si.on_wait and len(si.on_wait) > 1:
                waits = list(si.on_wait)
                for w in waits[:-1]:
                    nop = mybir.InstNoOp(
                        name=self.nc.get_next_instruction_name()
                    )
                    nop.engine = inst.engine
                    nop.sync_info = mybir.SyncInfo(on_wait=[w], on_update=[])
                    _orig_add(self, nop)
                inst.sync_info = mybir.SyncInfo(
                    on_wait=[waits[-1]], on_update=list(si.on_update or [])
                )
            _orig_add(self, inst)

        _add_instruction._split_waits_patch = True
        tile.TileContext._add_instruction = _add_instruction

    if not getattr(tile.TileContext._drain_and_barrier, "_split_waits_patch", False):
        # core_v3 walrus rejects a Drain carrying >1 sync wait ("Too many sync
        # wait commands"); put each wait on its own in-order SP nop instead.
        def _drain_and_barrier(self, tick_clock, wait_clock):
            nc = self.nc
            gc = tick_clock.global_clock
            for p in range(N_PROCS):
                t = gc[p]
                if t == 0:
                    continue
                vc = VectorClock([t if i == p else 0 for i in range(N_PROCS)])
                n = nc.sync.nop()
                wait_clock.add_sem_waits(n.ins, ScopedClock({None: vc}))
            nc.sync.drain()
            nc.all_engine_barrier()
            popped = nc._tile_sem_poison_stack.pop()
            assert popped is self._sem_poison
            nc.clear_and_free_semaphores(list(self.sems.allocated().values()))

        _drain_and_barrier._split_waits_patch = True
        tile.TileContext._drain_and_barrier = _drain_and_barrier

    from concourse import bass_utils
    bass_utils.upload_artifacts = lambda tmpdir: tmpdir
    _COMPAT_DONE = True


# ---------------------------------------------------------------------------
# host routing — bitwise replication of the reference (jax on CPU)
# ---------------------------------------------------------------------------
def _route(xt_f32, w_router):
    import jax
    import jax.numpy as jnp
    from jax import lax

    cpu = jax.devices("cpu")[0]

    def sinkhorn(cost, tol=1e-4):
        cost = jnp.exp(cost)
        T, E_ = cost.shape
        eps = 1e-8

        def cond(state):
            _, _, err = state
            return err > tol

        def body(state):
            d0, d1, _ = state
            d0n = (1.0 / T) / (cost @ d1 + eps)
            d1n = (1.0 / E_) / (d0n @ cost + eps)
            return (d0n, d1n, jnp.mean(jnp.abs(d1 - d1n)))

        init = (jnp.ones((T,), cost.dtype), jnp.ones((E_,), cost.dtype),
                jnp.asarray(1e9, cost.dtype))
        d0, d1, _ = lax.while_loop(cond, body, init)
        return d1 * cost * d0[:, None]

    with jax.default_device(cpu):
        xt_j = jnp.asarray(xt_f32)
        logits = xt_j @ jnp.asarray(w_router)
        norm = sinkhorn(logits.astype(jnp.float32))
        _, indices = lax.top_k(norm, 2)
        scores = jnp.take_along_axis(jax.nn.sigmoid(logits), indices, axis=1)
        return np.asarray(indices), np.asarray(scores)


# ---------------------------------------------------------------------------
# device kernel
# ---------------------------------------------------------------------------
_BUILD_CACHE = {}
LAST_EXEC_NS = None
LAST_RESULTS = None


def _build_nc(C):
    """Bass program for one core: dense FFN over C pre-gathered rows."""
    import concourse.bass as bass
    import concourse.mybir as mybir
    import concourse.tile as tile

    assert C % P == 0
    KC = H // P            # 4  k-chunks over hidden
    FC = F // P            # 16 f-chunks over ffn
    bf16 = mybir.dt.bfloat16
    f32 = mybir.dt.float32

    # token panels: full PANELs plus one remainder panel (multiple of 128)
    panels = []
    off = 0
    while off < C:
        w = min(PANEL, C - off)
        panels.append((off, w))
        off += w

    nc = bass.Bass()
    xt_d = nc.dram_tensor("xt", [P, KC, C], bf16, kind="ExternalInput")
    w1_d = nc.dram_tensor("w1", [P, KC, F], bf16, kind="ExternalInput")
    w2_d = nc.dram_tensor("w2", [P, FC, H], bf16, kind="ExternalInput")
    g_d = nc.dram_tensor("g", [P, C // P], f32, kind="ExternalInput")
    y_d = nc.dram_tensor("y", [C // P, P, H], f32, kind="ExternalOutput")

    with tile.TileContext(nc) as tc:
        with (
            tc.tile_pool(name="wpool", bufs=1) as wp,
            tc.tile_pool(name="xpool", bufs=4) as xp,
            tc.tile_pool(name="hpool", bufs=3) as hp,
            tc.tile_pool(name="opool", bufs=4) as op,
            tc.tile_pool(name="psum", bufs=4, space="PSUM") as pp,
        ):
            # HAM warm-up: ~16 dummy matmuls on zeroed SBUF while the input
            # DMAs are in flight. The PE clock gate needs ~3.4us of sustained
            # activity to open (1.2 -> 2.4 GHz); burn that during the startup
            # DMA window instead of during the first real matmuls.
            warm_sb = wp.tile([P, PANEL], bf16)
            nc.vector.memset(warm_sb, 0)
            warm_ps = pp.tile([P, PANEL], f32, tag="ps1")
            for _ in range(20):
                nc.tensor.matmul(
                    warm_ps, warm_sb[:, :P], warm_sb, start=True, stop=True
                )

            # first panel's tokens before the weights: PE needs x0 + w1 to
            # start; chunked DMAs land on parallel HW queues.
            p0_off, p0_w = panels[0]
            x0_sb = xp.tile([P, KC, PANEL], bf16, tag="x")
            x0_dma = None
            for kc in range(KC):
                x0_dma = nc.sync.dma_start(
                    out=x0_sb[:, kc, :p0_w],
                    in_=xt_d[:, kc, p0_off:p0_off + p0_w],
                )

            w1_sb = wp.tile([P, KC, F], bf16)
            for kc in range(KC):
                for hf in range(3):
                    lo = hf * 688
                    hi = min(F, lo + 688)
                    nc.sync.dma_start(
                        out=w1_sb[:, kc, lo:hi],
                        in_=w1_d[:, kc, lo:hi],
                    )
            # w2 isn't needed until the first phase-2 (~28us in); keep its
            # 2MB off the startup queues until the first x panel has landed.
            w2_sb = wp.tile([P, FC, H], bf16)
            for q in range(4):
                w2_dma = nc.sync.dma_start(
                    out=w2_sb[:, q * 4:(q + 1) * 4, :],
                    in_=w2_d[:, q * 4:(q + 1) * 4, :],
                )
                tile.add_dep_helper(
                    w2_dma.ins, x0_dma.ins, sync=True,
                    reason="delay w2 load past x0",
                )
            g_sb = wp.tile([P, C // P], f32)
            nc.sync.dma_start(out=g_sb, in_=g_d[:, :])

            for ip, (poff, pw) in enumerate(panels):
                tch_n = pw // P
                if ip == 0:
                    x_sb = x0_sb
                else:
                    x_sb = xp.tile([P, KC, PANEL], bf16, tag="x")
                    nc.sync.dma_start(
                        out=x_sb[:, :, :pw], in_=xt_d[:, :, poff:poff + pw]
                    )
                h1_sb = hp.tile([P, FC, PANEL], bf16, tag="h1")
                for fc in range(FC):
                    ps = pp.tile([P, PANEL], f32, tag="ps1")
                    for kc in range(KC):
                        nc.tensor.matmul(
                            ps[:, :pw],
                            w1_sb[:, kc, fc * P:(fc + 1) * P],
                            x_sb[:, kc, :pw],
                            start=(kc == 0),
                            stop=(kc == KC - 1),
                        )
                    nc.scalar.activation(
                        out=h1_sb[:, fc, :pw], in_=ps[:, :pw],
                        func=mybir.ActivationFunctionType.Silu,
                    )
                for tch in range(tch_n):
                    ps2 = pp.tile([P, H], f32, tag="ps2")
                    for fc in range(FC):
                        nc.tensor.matmul(
                            ps2,
                            h1_sb[:, fc, tch * P:(tch + 1) * P],
                            w2_sb[:, fc, :],
                            start=(fc == 0),
                            stop=(fc == FC - 1),
                        )
                    o_sb = op.tile([P, H], f32, tag="o")
                    j = poff // P + tch
                    nc.vector.tensor_scalar_mul(o_sb, ps2, g_sb[:, j:j + 1])
                    nc.sync.dma_start(out=y_d[j], in_=o_sb)
    return nc


def _pack_core(xt_f32, toks, gates, w1_e_bf, w2_e_bf, C):
    n = len(toks)
    xr = np.zeros((C, H), _BF16)
    xr[:n] = xt_f32[toks].astype(_BF16)
    # [C,H] -> [H,C] -> [KC,P,C] -> [P,KC,C]
    xt_pack = np.ascontiguousarray(
        xr.T.reshape(H // P, P, C).transpose(1, 0, 2)
    )
    g = np.zeros((C,), np.float32)
    g[:n] = gates
    g_pack = np.ascontiguousarray(g.reshape(C // P, P).T)
    return {"xt": xt_pack, "w1": w1_e_bf, "w2": w2_e_bf, "g": g_pack}


def kernel(input, w_router, w1, w2):
    global LAST_EXEC_NS, LAST_RESULTS
    import os

    _install_compat()
    from concourse.bass_utils import run_bass_kernel_spmd

    x = np.asarray(input, dtype=np.float32)
    w_router = np.asarray(w_router, dtype=np.float32)
    w1 = np.asarray(w1, dtype=np.float32)
    w2 = np.asarray(w2, dtype=np.float32)
    s, b, h = x.shape
    T = s * b
    xt = np.ascontiguousarray(x.reshape(T, h))

    indices, scores = _route(xt, w_router)

    # per-expert (token, gate) lists
    tok_lists = []
    gate_lists = []
    for e in range(E):
        toks = []
        gs = []
        for k in range(2):
            sel = np.nonzero(indices[:, k] == e)[0]
            toks.append(sel)
            gs.append(scores[sel, k])
        tok_lists.append(np.concatenate(toks))
        gate_lists.append(np.concatenate(gs).astype(np.float32))

    max_half = max((len(t) + 1) // 2 for t in tok_lists)
    C = max(DEFAULT_C, ((max_half + P - 1) // P) * P)

    if C not in _BUILD_CACHE:
        _BUILD_CACHE[C] = _build_nc(C)
    nc = _BUILD_CACHE[C]

    # weights per expert, packed [P, KC, F] / [P, FC, H] bf16
    w1_packs = [
        np.ascontiguousarray(
            w1[e].astype(_BF16).reshape(H // P, P, F).transpose(1, 0, 2)
        )
        for e in range(E)
    ]
    w2_packs = [
        np.ascontiguousarray(
            w2[e].astype(_BF16).reshape(F // P, P, H).transpose(1, 0, 2)
        )
        for e in range(E)
    ]

    in_maps = []
    core_toks = []
    for c in range(NCORES):
        e = c // 2
        toks_e = tok_lists[e]
        gates_e = gate_lists[e]
        half = (len(toks_e) + 1) // 2
        if c % 2 == 0:
            toks, gs = toks_e[:half], gates_e[:half]
        else:
            toks, gs = toks_e[half:], gates_e[half:]
        assert len(toks) <= C
        core_toks.append(toks)
        in_maps.append(_pack_core(xt, toks, gs, w1_packs[e], w2_packs[e], C))

    trace = bool(int(os.environ.get("BASS_MOE_TRACE", "0")))
    # The axon/NRT path can throw a transient NRT_EXEC_UNIT_UNRECOVERABLE;
    # the dispatch is a pure function of in_maps, so retrying is safe.
    last_err = None
    for attempt in range(3):
        try:
            res = run_bass_kernel_spmd(
                nc, in_maps, list(range(NCORES)), trace=trace
            )
            break
        except Exception as e:
            last_err = e
            print(f"kernel: device run attempt {attempt} failed: {e}",
                  file=sys.stderr)
            import time as _time
            _time.sleep(2.0)
    else:
        raise last_err
    LAST_EXEC_NS = res.exec_time_ns
    LAST_RESULTS = res

    out = np.zeros((T, H), np.float32)
    for c in range(NCORES):
        y = res.results[c]["y"].reshape(-1, H)
        toks = core_toks[c]
        out[toks] += y[: len(toks)]
    return out.reshape(s, b, h)


# revision 22
# speedup vs baseline: 1.0308x; 1.0027x over previous
"""MoE top-2 routing kernel for Trainium2 (8 NeuronCores, expert-parallel).

Strategy
--------
Host (cheap, 16384x4-sized math): router logits, sinkhorn, top-2 indices and
sigmoid gates — computed with jax on CPU, replicating the reference bitwise.
Tokens are permuted by expert on the host while sharding: each of the 8 cores
owns half of one expert's (token, gate) list plus that expert's W1/W2 (bf16).

Device (the heavy ~17 GFLOP/core): dense FFN over the pre-gathered tokens in
feature-major layout, weight-stationary matmuls from SBUF:
    h1T = silu(W1_chunk.T @ xT)      [F-major]
    y   = gate * (h1T_chunk.T @ W2)  [token-major out]
Host scatter-adds the two expert contributions per token (no duplicates per
core, so fancy-index += is safe).
"""
import sys
import types

import numpy as np
import ml_dtypes

H = 512
F = 2048
E = 4
P = 128
PANEL = 512
NCORES = 8
T_TOTAL = 16384
DEFAULT_C = 4224  # rows (token,expert pairs) per core, multiple of 128

_BF16 = ml_dtypes.bfloat16


# ---------------------------------------------------------------------------
# compat shims (axon image): NTFF hook module + core_v3 drain-wait splitting
# ---------------------------------------------------------------------------
_COMPAT_DONE = False


def _install_compat():
    global _COMPAT_DONE
    if _COMPAT_DONE:
        return
    if "antenv.axon_hooks" not in sys.modules:
        mod = types.ModuleType("antenv.axon_hooks")
        try:
            from trn_agent_boot.trn_boot import _ntff_profile_via_ctypes
            _hook = _ntff_profile_via_ctypes("/opt/axon/libaxon_pjrt.so")
        except Exception:
            _hook = None
        mod.get_axon_ntff_profile_hook = lambda: _hook
        mod.set_axon_ntff_profile_hook = lambda h: None
        sys.modules["antenv.axon_hooks"] = mod

    import concourse.mybir as mybir
    import concourse.tile as tile
    from bass_rust import VectorClock, ScopedClock, N_PROCS

    if not getattr(tile.TileContext._add_instruction, "_split_waits_patch", False):
        # This walrus build accepts at most ONE sync wait per instruction
        # ("Too many sync wait commands"). Split extras onto single-wait
        # nops on the same engine, inserted immediately before. Stalling the
        # engine at the same program point is strictly stronger than the
        # per-instruction wait, and every waited-on producer is issued
        # earlier in program order, so this cannot deadlock.
        _orig_add = tile.TileContext._add_instruction

        def _add_instruction(self, inst):
            si = inst.sync_info
            if si is not None and si.on_wait and len(si.on_wait) > 1:
                waits = list(si.on_wait)
                for w in waits[:-1]:
                    nop = mybir.InstNoOp(
                        name=self.nc.get_next_instruction_name()
                    )
                    nop.engine = inst.engine
                    nop.sync_info = mybir.SyncInfo(on_wait=[w], on_update=[])
                    _orig_add(self, nop)
                inst.sync_info = mybir.SyncInfo(
                    on_wait=[waits[-1]], on_update=list(si.on_update or [])
                )
            _orig_add(self, inst)

        _add_instruction._split_waits_patch = True
        tile.TileContext._add_instruction = _add_instruction

    if not getattr(tile.TileContext._drain_and_barrier, "_split_waits_patch", False):
        # core_v3 walrus rejects a Drain carrying >1 sync wait ("Too many sync
        # wait commands"); put each wait on its own in-order SP nop instead.
        def _drain_and_barrier(self, tick_clock, wait_clock):
            nc = self.nc
            gc = tick_clock.global_clock
            for p in range(N_PROCS):
                t = gc[p]
                if t == 0:
                    continue
                vc = VectorClock([t if i == p else 0 for i in range(N_PROCS)])
                n = nc.sync.nop()
                wait_clock.add_sem_waits(n.ins, ScopedClock({None: vc}))
            nc.sync.drain()
            nc.all_engine_barrier()
            popped = nc._tile_sem_poison_stack.pop()
            assert popped is self._sem_poison
            nc.clear_and_free_semaphores(list(self.sems.allocated().values()))

        _drain_and_barrier._split_waits_patch = True
        tile.TileContext._drain_and_barrier = _drain_and_barrier

    from concourse import bass_utils
    bass_utils.upload_artifacts = lambda tmpdir: tmpdir
    _COMPAT_DONE = True


# ---------------------------------------------------------------------------
# host routing — bitwise replication of the reference (jax on CPU)
# ---------------------------------------------------------------------------
def _route(xt_f32, w_router):
    import jax
    import jax.numpy as jnp
    from jax import lax

    cpu = jax.devices("cpu")[0]

    def sinkhorn(cost, tol=1e-4):
        cost = jnp.exp(cost)
        T, E_ = cost.shape
        eps = 1e-8

        def cond(state):
            _, _, err = state
            return err > tol

        def body(state):
            d0, d1, _ = state
            d0n = (1.0 / T) / (cost @ d1 + eps)
            d1n = (1.0 / E_) / (d0n @ cost + eps)
            return (d0n, d1n, jnp.mean(jnp.abs(d1 - d1n)))

        init = (jnp.ones((T,), cost.dtype), jnp.ones((E_,), cost.dtype),
                jnp.asarray(1e9, cost.dtype))
        d0, d1, _ = lax.while_loop(cond, body, init)
        return d1 * cost * d0[:, None]

    with jax.default_device(cpu):
        xt_j = jnp.asarray(xt_f32)
        logits = xt_j @ jnp.asarray(w_router)
        norm = sinkhorn(logits.astype(jnp.float32))
        _, indices = lax.top_k(norm, 2)
        scores = jnp.take_along_axis(jax.nn.sigmoid(logits), indices, axis=1)
        return np.asarray(indices), np.asarray(scores)


# ---------------------------------------------------------------------------
# device kernel
# ---------------------------------------------------------------------------
_BUILD_CACHE = {}
LAST_EXEC_NS = None
LAST_RESULTS = None


def _build_nc(C, C_comp=None):
    """Bass program for one core: dense FFN over C pre-gathered rows.

    C is the allocated capacity (multiple of 128); C_comp <= C is how many
    rows phase 1 actually computes (the max real row count over cores).
    Phase 2 still runs ceil(C_comp/128) full 128-token chunks; h1 columns
    past C_comp in the last chunk are uninitialized, which only pollutes
    output rows past C_comp — rows the host never reads (the runtime
    pre-zeroes output buffers).
    """
    import concourse.bass as bass
    import concourse.mybir as mybir
    import concourse.tile as tile

    assert C % P == 0
    if C_comp is None:
        C_comp = C
    assert C_comp <= C
    KC = H // P            # 4  k-chunks over hidden
    FC = F // P            # 16 f-chunks over ffn
    bf16 = mybir.dt.bfloat16
    f32 = mybir.dt.float32

    # token panels: full PANELs plus one exact-width remainder panel
    panels = []
    off = 0
    while off < C_comp:
        w = min(PANEL, C_comp - off)
        panels.append((off, w))
        off += w

    nc = bass.Bass()
    xt_d = nc.dram_tensor("xt", [P, KC, C], bf16, kind="ExternalInput")
    w1_d = nc.dram_tensor("w1", [P, KC, F], bf16, kind="ExternalInput")
    w2_d = nc.dram_tensor("w2", [P, FC, H], bf16, kind="ExternalInput")
    g_d = nc.dram_tensor("g", [P, C // P], f32, kind="ExternalInput")
    y_d = nc.dram_tensor("y", [C // P, P, H], f32, kind="ExternalOutput")

    with tile.TileContext(nc) as tc:
        with (
            tc.tile_pool(name="wpool", bufs=1) as wp,
            tc.tile_pool(name="xpool", bufs=4) as xp,
            tc.tile_pool(name="hpool", bufs=3) as hp,
            tc.tile_pool(name="opool", bufs=4) as op,
            tc.tile_pool(name="psum", bufs=4, space="PSUM") as pp,
        ):
            # HAM warm-up: ~16 dummy matmuls on zeroed SBUF while the input
            # DMAs are in flight. The PE clock gate needs ~3.4us of sustained
            # activity to open (1.2 -> 2.4 GHz); burn that during the startup
            # DMA window instead of during the first real matmuls.
            warm_sb = wp.tile([P, PANEL], bf16)
            nc.vector.memset(warm_sb, 0)
            warm_ps = pp.tile([P, PANEL], f32, tag="ps1")
            for _ in range(20):
                nc.tensor.matmul(
                    warm_ps, warm_sb[:, :P], warm_sb, start=True, stop=True
                )

            # first panel's tokens before the weights: PE needs x0 + w1 to
            # start; chunked DMAs land on parallel HW queues.
            p0_off, p0_w = panels[0]
            x0_sb = xp.tile([P, KC, PANEL], bf16, tag="x")
            x0_dma = None
            for kc in range(KC):
                x0_dma = nc.sync.dma_start(
                    out=x0_sb[:, kc, :p0_w],
                    in_=xt_d[:, kc, p0_off:p0_off + p0_w],
                )

            w1_sb = wp.tile([P, KC, F], bf16)
            for kc in range(KC):
                for hf in range(3):
                    lo = hf * 688
                    hi = min(F, lo + 688)
                    nc.sync.dma_start(
                        out=w1_sb[:, kc, lo:hi],
                        in_=w1_d[:, kc, lo:hi],
                    )
            # w2 isn't needed until the first phase-2 (~28us in); keep its
            # 2MB off the startup queues until the first x panel has landed.
            w2_sb = wp.tile([P, FC, H], bf16)
            for q in range(4):
                w2_dma = nc.sync.dma_start(
                    out=w2_sb[:, q * 4:(q + 1) * 4, :],
                    in_=w2_d[:, q * 4:(q + 1) * 4, :],
                )
                tile.add_dep_helper(
                    w2_dma.ins, x0_dma.ins, sync=True,
                    reason="delay w2 load past x0",
                )
            g_sb = wp.tile([P, C // P], f32)
            nc.sync.dma_start(out=g_sb, in_=g_d[:, :])

            for ip, (poff, pw) in enumerate(panels):
                tch_n = (pw + P - 1) // P
                if ip == 0:
                    x_sb = x0_sb
                else:
                    x_sb = xp.tile([P, KC, PANEL], bf16, tag="x")
                    nc.sync.dma_start(
                        out=x_sb[:, :, :pw], in_=xt_d[:, :, poff:poff + pw]
                    )
                h1_sb = hp.tile([P, FC, PANEL], bf16, tag="h1")
                for fc in range(FC):
                    ps = pp.tile([P, PANEL], f32, tag="ps1")
                    for kc in range(KC):
                        nc.tensor.matmul(
                            ps[:, :pw],
                            w1_sb[:, kc, fc * P:(fc + 1) * P],
                            x_sb[:, kc, :pw],
                            start=(kc == 0),
                            stop=(kc == KC - 1),
                        )
                    nc.scalar.activation(
                        out=h1_sb[:, fc, :pw], in_=ps[:, :pw],
                        func=mybir.ActivationFunctionType.Silu,
                    )
                for tch in range(tch_n):
                    ps2 = pp.tile([P, H], f32, tag="ps2")
                    for fc in range(FC):
                        nc.tensor.matmul(
                            ps2,
                            h1_sb[:, fc, tch * P:(tch + 1) * P],
                            w2_sb[:, fc, :],
                            start=(fc == 0),
                            stop=(fc == FC - 1),
                        )
                    o_sb = op.tile([P, H], f32, tag="o")
                    j = poff // P + tch
                    nc.vector.tensor_scalar_mul(o_sb, ps2, g_sb[:, j:j + 1])
                    nc.sync.dma_start(out=y_d[j], in_=o_sb)
    return nc


def _pack_core(xt_f32, toks, gates, w1_e_bf, w2_e_bf, C):
    n = len(toks)
    xr = np.zeros((C, H), _BF16)
    xr[:n] = xt_f32[toks].astype(_BF16)
    # [C,H] -> [H,C] -> [KC,P,C] -> [P,KC,C]
    xt_pack = np.ascontiguousarray(
        xr.T.reshape(H // P, P, C).transpose(1, 0, 2)
    )
    g = np.zeros((C,), np.float32)
    g[:n] = gates
    g_pack = np.ascontiguousarray(g.reshape(C // P, P).T)
    return {"xt": xt_pack, "w1": w1_e_bf, "w2": w2_e_bf, "g": g_pack}


def kernel(input, w_router, w1, w2):
    global LAST_EXEC_NS, LAST_RESULTS
    import os

    _install_compat()
    from concourse.bass_utils import run_bass_kernel_spmd

    x = np.asarray(input, dtype=np.float32)
    w_router = np.asarray(w_router, dtype=np.float32)
    w1 = np.asarray(w1, dtype=np.float32)
    w2 = np.asarray(w2, dtype=np.float32)
    s, b, h = x.shape
    T = s * b
    xt = np.ascontiguousarray(x.reshape(T, h))

    indices, scores = _route(xt, w_router)

    # per-expert (token, gate) lists
    tok_lists = []
    gate_lists = []
    for e in range(E):
        toks = []
        gs = []
        for k in range(2):
            sel = np.nonzero(indices[:, k] == e)[0]
            toks.append(sel)
            gs.append(scores[sel, k])
        tok_lists.append(np.concatenate(toks))
        gate_lists.append(np.concatenate(gs).astype(np.float32))

    max_half = max((len(t) + 1) // 2 for t in tok_lists)
    C = max(DEFAULT_C, ((max_half + P - 1) // P) * P)
    C_comp = min(C, ((max_half + 1) // 2) * 2)  # exact rows, 2-aligned

    key = (C, C_comp)
    if key not in _BUILD_CACHE:
        _BUILD_CACHE[key] = _build_nc(C, C_comp)
    nc = _BUILD_CACHE[key]

    # weights per expert, packed [P, KC, F] / [P, FC, H] bf16
    w1_packs = [
        np.ascontiguousarray(
            w1[e].astype(_BF16).reshape(H // P, P, F).transpose(1, 0, 2)
        )
        for e in range(E)
    ]
    w2_packs = [
        np.ascontiguousarray(
            w2[e].astype(_BF16).reshape(F // P, P, H).transpose(1, 0, 2)
        )
        for e in range(E)
    ]

    in_maps = []
    core_toks = []
    for c in range(NCORES):
        e = c // 2
        toks_e = tok_lists[e]
        gates_e = gate_lists[e]
        half = (len(toks_e) + 1) // 2
        if c % 2 == 0:
            toks, gs = toks_e[:half], gates_e[:half]
        else:
            toks, gs = toks_e[half:], gates_e[half:]
        assert len(toks) <= C
        core_toks.append(toks)
        in_maps.append(_pack_core(xt, toks, gs, w1_packs[e], w2_packs[e], C))

    trace = bool(int(os.environ.get("BASS_MOE_TRACE", "0")))
    # The axon/NRT path can throw a transient NRT_EXEC_UNIT_UNRECOVERABLE;
    # the dispatch is a pure function of in_maps, so retrying is safe.
    last_err = None
    for attempt in range(3):
        try:
            res = run_bass_kernel_spmd(
                nc, in_maps, list(range(NCORES)), trace=trace
            )
            break
        except Exception as e:
            last_err = e
            print(f"kernel: device run attempt {attempt} failed: {e}",
                  file=sys.stderr)
            import time as _time
            _time.sleep(2.0)
    else:
        raise last_err
    LAST_EXEC_NS = res.exec_time_ns
    LAST_RESULTS = res

    out = np.zeros((T, H), np.float32)
    for c in range(NCORES):
        y = res.results[c]["y"].reshape(-1, H)
        toks = core_toks[c]
        out[toks] += y[: len(toks)]
    return out.reshape(s, b, h)
